# revision 54
# baseline (speedup 1.0000x reference)
"""Multi-head attention (B=4, T=S=2048, E=1024, H=16) on 8 trn2 NeuronCores.

Sharding: core c handles batch b = c // 2 and head-half hh = c % 2
(8 of 16 heads).  Each core computes its heads' Q/K/V projections,
attention, and a partial output projection (contraction over its 512
e-dims).  The host sums the two partial outputs per batch and adds bo.

Key layout choices:
 - The host passes PRE-TRANSPOSED, PRE-CAST-bf16 activations and
   weights (q.T, k.T, v.T, Wq_slice.T, ...), so no on-chip transposes
   of x or W are needed and DMA bytes are halved: loads land directly
   in the [e_in partitions, t] layout the projections consume.
 - scores.T = kp @ qp.T is computed per head as [s, t] tiles
   (s on partitions), exp'd on ACT into bf16 pt tiles.  exp is the
   single largest engine load (~266us on ACT); the whole schedule is
   built to keep it streaming.
 - PV runs in the "natural" orientation: ctx[t, hd] = sum_s
   pt[s, t] * vp[s, hd], i.e. lhsT = pt (stationary), rhs = vp
   with an extra ones-column producing the softmax denominator in
   column 64.  Output columns per matmul are 65 instead of 512,
   which is ~2x fewer PE cycles for the PV stage.  The 8 (head,
   t-chunk) accumulation regions share PSUM banks, so the tile is
   zeroed via single-group ones⊗zero matmuls and PV runs start=False
   (a start=True per region would clear the whole bank's has_written
   and wipe sibling regions — hardware-verified failure mode).
 - Normalization is a per-partition (per-t) reciprocal multiply on
   DVE, no cross-partition broadcast needed.
 - ctx is transposed back (PE transposes) only for the tiny
   [2048 x 512] normalized context, feeding the output projection.
 - t-block 0 runs a one-behind-PV software pipeline (scores of unit w
   stream while PV of unit w-1 + v-projection + k-row fills retire on
   PE), so exp starts ~15us in; t-blocks 1-3 run same-unit PV with the
   previous block's ctx-transposes + output projection and the next
   block's q-projection emitted into their ACT-bound windows.
"""

import numpy as np

import concourse.bass as bass
import concourse.mybir as mybir
import concourse.tile as tile
from concourse.bass_utils import run_bass_kernel_spmd
from concourse.masks import make_identity

F32 = mybir.dt.float32
BF16 = mybir.dt.bfloat16

B, T, E = 4, 2048, 1024
H = 16  # global heads
HL = 8  # heads per core (local)
HD = 64  # head dim
EL = HL * HD  # 512, e-dims per core
N_CORES = 8

_CACHED = {}


def legalize_waits(nc, cap=1):
    """Hoist semaphore waits so no instruction carries more than `cap`.

    The cayman 64B ISA instruction format has a single wait slot
    (NEURON_ISA_TPB_EVENTS); this container's walrus rejects instructions
    with more attached waits ("Too many sync wait commands").  Tile's sem
    assignment freely attaches several, so we split the excess onto
    standalone InstEventSemaphore carriers (exactly what raw-bass
    wait_ge emits) on the same engine, immediately before.
    """
    import bass_rust

    # Pass 1: statically-known final value of every semaphore (sum of all
    # attached increments) — needed to replace the tail RANGE_CLEAR (an
    # InstISA opcode this walrus can't codegen) with sem-dec updates.
    totals = {}
    names = {}
    for f in nc.m.functions:
        for bb in f.blocks:
            for ins in bb.instructions:
                si = ins.sync_info
                if si is None:
                    continue
                for u in si.on_update or []:
                    if u.sync_type == "semaphore":
                        sign = 1 if u.update_mode in ("sem-inc", "sem-add-imm") else -1
                        totals[u.id] = totals.get(u.id, 0) + sign * u.update_value
                        names[u.id] = u.ant_name

    n = 0
    for f in nc.m.functions:
        for bb in f.blocks:
            insts = bb.instructions
            out = []
            changed = False
            for ins in insts:
                if type(ins).__name__ == "InstISA" and "RANGE_CLEAR" in str(ins):
                    import re

                    m = re.search(r"range_first=(\d+) range_last=(\d+)", str(ins))
                    first, last = int(m.group(1)), int(m.group(2))
                    for sid in range(first, last + 1):
                        tot = totals.get(sid, 0)
                        if tot == 0:
                            continue
                        ev = mybir.InstEventSemaphore(name=f"I-LC{n}", ins=[], outs=[])
                        n += 1
                        ev.engine = ins.engine
                        ev.sync_info = bass_rust.SyncInfo(
                            on_wait=[],
                            on_update=[
                                bass_rust.SyncUpdate(
                                    sync_type="semaphore",
                                    id=sid,
                                    ant_name=names.get(sid, f"sem{sid}"),
                                    update_mode="sem-sub-imm",
                                    update_value=tot,
                                    update_reg=None,
                                )
                            ],
                        )
                        out.append(ev)
                    changed = True
                    continue
                si = ins.sync_info
                ws = list(si.on_wait) if (si is not None and si.on_wait) else []
                if len(ws) > cap:
                    for w in ws[: len(ws) - cap]:
                        ev = mybir.InstEventSemaphore(
                            name=f"I-LW{n}", ins=[], outs=[]
                        )
                        n += 1
                        ev.engine = ins.engine
                        ev.sync_info = bass_rust.SyncInfo(
                            on_wait=[w], on_update=[]
                        )
                        out.append(ev)
                    si.on_wait = ws[len(ws) - cap :]
                    changed = True
                out.append(ins)
            if changed:
                insts[:] = out
    return n


def build_program():
    nc = bass.Bass()

    # Activations/weights arrive pre-transposed AND pre-cast to bf16 on the
    # host: halves the DMA bytes and removes the cast from the DMA path.
    qtd = nc.declare_dram_parameter("qT", [E, T], BF16, isOutput=False)
    ktd = nc.declare_dram_parameter("kT", [E, T], BF16, isOutput=False)
    vtd = nc.declare_dram_parameter("vT", [E, T], BF16, isOutput=False)
    wqtd = nc.declare_dram_parameter("wqT", [E, EL], BF16, isOutput=False)
    wktd = nc.declare_dram_parameter("wkT", [E, EL], BF16, isOutput=False)
    wvtd = nc.declare_dram_parameter("wvT", [E, EL], BF16, isOutput=False)
    wotd = nc.declare_dram_parameter("woT", [EL, E], BF16, isOutput=False)
    bqd = nc.declare_dram_parameter("bq", [EL], F32, isOutput=False)
    bkd = nc.declare_dram_parameter("bk", [EL], F32, isOutput=False)
    bvd = nc.declare_dram_parameter("bv", [EL], F32, isOutput=False)
    outd = nc.declare_dram_parameter("outT", [E, T], BF16, isOutput=True)

    with tile.TileContext(nc, pool_alloc_mode="queue") as tc:
        with (
            tc.tile_pool(name="singles", bufs=1) as singles,
            tc.tile_pool(name="xin", bufs=4) as xin,
            tc.tile_pool(name="acts", bufs=1) as acts,
            tc.tile_pool(name="pt", bufs=1) as ptp,
            tc.tile_pool(name="norm", bufs=8) as normp,
            tc.tile_pool(name="osb", bufs=4) as osbp,
            tc.tile_pool(name="proj_ps", bufs=2, space="PSUM") as proj_ps,
            tc.tile_pool(name="sc_ps", bufs=2, space="PSUM") as sc_ps,
            tc.tile_pool(name="ctx_ps", bufs=1, space="PSUM") as ctx_ps,
        ):
            # ---------------- prologue: weights / biases / consts ----------
            ident = singles.tile([128, 128], BF16)
            make_identity(nc, ident)

            # Transposed bf16 weights, loaded directly (host pre-transposed):
            #   wqT[p, c, o] = Wq_c[o, c*128 + p]   (c,p) = e_in in [0,1024)
            wqT = singles.tile([128, 8, EL], BF16)
            wkT = singles.tile([128, 8, EL], BF16)
            wvT = singles.tile([128, 8, EL], BF16)
            # woT[p, c, o] = Wo_c[o, c*128 + p]     (c,p) = local e in [0,512)
            woT = singles.tile([128, 4, E], BF16)

            # k-path DMAs first: the whole k projection gates attention.
            # The four ramp-critical loads go out on FOUR different DGE
            # queues (gpsimd/SWDGE + 3 HWDGE engines) so their descriptor-gen
            # and latency chains overlap instead of serializing.
            nc.gpsimd.dma_start(
                out=wkT, in_=wktd.rearrange("(c p) o -> p c o", p=128)
            )
            bq_sb = singles.tile([128, 4], F32)
            bk_sb = singles.tile([128, 4], F32)
            nc.gpsimd.dma_start(out=bk_sb, in_=bkd.rearrange("(c p) -> p c", p=128))
            nc.gpsimd.dma_start(out=bq_sb, in_=bqd.rearrange("(c p) -> p c", p=128))
            ones_col = singles.tile([1, 128], BF16)
            nc.vector.memset(ones_col, 1.0)
            zero_row = singles.tile([1, 512], BF16)
            nc.vector.memset(zero_row, 0.0)
            bv_sb = singles.tile([1, EL], BF16)

            # ---------------- activations / projections --------------------
            # qpT[p, j, t] = qp[t, j*128 + p]  (pair j: head 2j at p<64)
            qpT = acts.tile([128, 4, T], BF16)
            kpT = acts.tile([128, 4, T], BF16)
            # vp_ext[p, s, h*65 + d] = vp[s*128 + p, h*64 + d]; col h*65+64 = 1
            vp_ext = acts.tile([128, 16, HL * 65], BF16)

            def load_x_chunk(xd, tb):
                """x.T [E, T] bf16 DRAM chunk tb -> bf16 SBUF [128, 8, 512]."""
                xc = xin.tile([128, 8, 512], BF16, tag="xin")
                nc.gpsimd.dma_start(
                    out=xc,
                    in_=xd[:, tb * 512 : (tb + 1) * 512].rearrange(
                        "(c p) t -> p c t", p=128
                    ),
                )
                return xc

            def proj_qk_group(xc, xpT, b_sb, wT, c, tb):
                """One (e_out chunk c, t-block tb) projection psum group."""
                ps = proj_ps.tile([128, 512], F32, tag="proj")
                for e in range(8):
                    nc.tensor.matmul(
                        ps,
                        lhsT=wT[:, e, c * 128 : (c + 1) * 128],
                        rhs=xc[:, e, :],
                        start=(e == 0),
                        stop=(e == 7),
                    )
                nc.vector.tensor_scalar_add(
                    out=xpT[:, c, tb * 512 : (tb + 1) * 512],
                    in0=ps,
                    scalar1=b_sb[:, c : c + 1],
                )

            def proj_qk_chunk(xc, xpT, b_sb, wT, tb):
                """One t-block of the q/k projection: fills xpT[:, :, tsl]."""
                for c in range(4):
                    proj_qk_group(xc, xpT, b_sb, wT, c, tb)

            def proj_v_schunk(vc, s):
                """One 128-row s-chunk of the v projection -> vp_ext[:, s, :].

                vc is the loaded vT chunk covering s; u = s % 4 indexes the
                128-slice within it.
                """
                u = s % 4
                ps = proj_ps.tile([128, 512], F32, tag="proj")
                for e in range(8):
                    nc.tensor.matmul(
                        ps,
                        lhsT=vc[:, e, u * 128 : (u + 1) * 128],
                        rhs=wvT[:, e, :],
                        start=(e == 0),
                        stop=False,
                    )
                # += ones ⊗ bv  (bias along the free dim)
                nc.tensor.matmul(ps, lhsT=ones_col, rhs=bv_sb, start=False, stop=True)
                nc.vector.memset(vp_ext[:, s, :], 1.0)
                nc.vector.tensor_copy(
                    out=vp_ext[:, s, :].rearrange("p (h x) -> p h x", x=65)[
                        :, :, 0:64
                    ],
                    in_=ps.rearrange("p (h d) -> p h d", d=64),
                )

            # k is projected c-row-wise (c = head-pair j): attention unit
            # (j, tb) only needs row c=j of kpT, so unit (0, 0) can start
            # right after row 0 — rows 1..3 are spread into tb0's units.
            # kT is loaded as one full tile (each c-row contracts over all T).
            ktf = xin.tile([128, 8, T], BF16, tag="ktf", bufs=1)
            # first k chunk rides the otherwise-idle SP/HWDGE queue so it
            # lands in parallel with wkT on the SWDGE queue
            nc.sync.dma_start(
                out=ktf[:, :, 0:512],
                in_=ktd[:, 0:512].rearrange("(c p) t -> p c t", p=128),
            )
            nc.gpsimd.dma_start(
                out=wqT, in_=wqtd.rearrange("(c p) o -> p c o", p=128)
            )
            qc = load_x_chunk(qtd, 0)
            nc.gpsimd.dma_start(
                out=ktf[:, :, 512:1024],
                in_=ktd[:, 512:1024].rearrange("(c p) t -> p c t", p=128),
            )
            nc.gpsimd.dma_start(
                out=ktf[:, :, 1024:1536],
                in_=ktd[:, 1024:1536].rearrange("(c p) t -> p c t", p=128),
            )
            nc.gpsimd.dma_start(
                out=ktf[:, :, 1536:2048],
                in_=ktd[:, 1536:2048].rearrange("(c p) t -> p c t", p=128),
            )
            nc.gpsimd.dma_start(
                out=wvT, in_=wvtd.rearrange("(c p) o -> p c o", p=128)
            )
            nc.gpsimd.dma_start(out=bv_sb, in_=bvd.rearrange("(o e) -> o e", o=1))
            # v chunks: chunk 0 early (its projection fills the ramp's
            # PE-idle gap); the rest needed from window 1 (~30us in)
            vcs = [load_x_chunk(vtd, i) for i in range(4)]

            # Minimal pre-attention PE work: one k-group and one q-group —
            # exactly what scores (j=0, tb=0, s-chunk 0) needs.  Everything
            # else rides just-in-time inside tb0's units.
            proj_qk_group(ktf[:, :, 0:512], kpT, bk_sb, wkT, 0, 0)
            proj_qk_group(qc, qpT, bq_sb, wqT, 0, 0)
            nc.gpsimd.dma_start(
                out=woT, in_=wotd.rearrange("(c p) o -> p c o", p=128)
            )

            def kg_thunk(c, tbk):
                def f():
                    proj_qk_group(
                        ktf[:, :, tbk * 512 : (tbk + 1) * 512],
                        kpT, bk_sb, wkT, c, tbk,
                    )
                return f

            def qg_thunk(c):
                def f():
                    proj_qk_group(qc, qpT, bq_sb, wqT, c, 0)
                return f

            # ---------------- attention ----------------------------------
            # normalized ctx, natural layout, per t-chunk of 128:
            # ctxn[t', e_local] for t = tb*512 + tc*128 + t'
            # assembled across the 4 j-units of a t-block.
            ctxn_sb = {}

            v_loaded = [None]

            pt_store = {}  # u -> list of 16 pt tiles
            ctx_store = {}  # u -> ctx psum tile

            def emit_zero(u):
                """Alloc unit u's ctx tile and zero it.  The 8 (head,
                t-chunk) PV accumulation regions interleave within its two
                banks; a start=True on hardware clears has_written for the
                WHOLE bank, wiping sibling regions, so instead: zero each
                bank with a single-group ones⊗zero matmul and run every PV
                matmul in accumulate mode (correct for any has_written
                state: bit=1 accumulates onto 0, bit=0 overwrites)."""
                ctx = ctx_ps.tile([128, 2, 512], F32, tag="ctx", name=f"ctx{u}")
                ctx_store[u] = ctx
                for hh in range(2):
                    nc.tensor.matmul(
                        ctx[:, hh, :], lhsT=ones_col, rhs=zero_row,
                        start=True, stop=True,
                    )

            def emit_scores(u, s):
                tb, j = divmod(u, 4)
                tsl = slice(tb * 512, (tb + 1) * 512)
                sc = sc_ps.tile([128, 1024], F32, tag="sc")
                nc.tensor.matmul(
                    sc[:, 0:512],
                    lhsT=kpT[0:64, j, s * 128 : (s + 1) * 128],
                    rhs=qpT[0:64, j, tsl],
                    start=True,
                    stop=True,
                )
                nc.tensor.matmul(
                    sc[:, 512:1024],
                    lhsT=kpT[64:128, j, s * 128 : (s + 1) * 128],
                    rhs=qpT[64:128, j, tsl],
                    start=True,
                    stop=True,
                )
                pt = ptp.tile(
                    [128, 1024], BF16, tag="pt", bufs=18, name=f"pt{u}_{s}"
                )
                nc.scalar.activation(
                    out=pt,
                    in_=sc,
                    func=mybir.ActivationFunctionType.Exp,
                    scale=0.125,
                )
                pt_store.setdefault(u, {})[s] = pt

            def emit_pv(u, s):
                tb, j = divmod(u, 4)
                ctx = ctx_store[u]
                pt = pt_store[u].pop(s)
                for hh in range(2):
                    for tc in range(4):
                        nc.tensor.matmul(
                            ctx[:, hh, tc * 128 : tc * 128 + 65],
                            lhsT=pt[
                                :, hh * 512 + tc * 128 : hh * 512 + (tc + 1) * 128
                            ],
                            rhs=vp_ext[
                                :, s, (2 * j + hh) * 65 : (2 * j + hh + 1) * 65
                            ],
                            start=False,
                            stop=(s == 15),
                            skip_group_check=True,
                        )

            def emit_norm(u):
                # normalize: column tc*128+64 of ctx[:, hh, :] holds denoms
                tb, j = divmod(u, 4)
                ctx = ctx_store.pop(u)
                recip = normp.tile(
                    [128, 2, 4, 1], F32, tag="recip", bufs=2, name=f"recip{u}"
                )
                nc.vector.reciprocal(
                    out=recip,
                    in_=ctx.rearrange("p h (tc x) -> p h tc x", x=128)[
                        :, :, :, 64:65
                    ],
                )
                for tc in range(4):
                    for hh in range(2):
                        hl = 2 * j + hh
                        nc.vector.tensor_scalar_mul(
                            out=ctxn_sb[tb, tc][:, hl * 64 : (hl + 1) * 64],
                            in0=ctx[:, hh, tc * 128 : tc * 128 + 64],
                            scalar1=recip[:, hh, tc, :],
                        )

            def emit_unit(j, tb, fills=()):
                """Scores+PV streaming with PV one s-step behind: zero(u)
                (which waits on norm(u-1)'s ctx slot, fresh DVE output) is
                emitted after the first scores, so the boundary stall hides
                behind the exp stream instead of blocking PE head-of-line."""
                u = 4 * tb + j
                fq = list(fills)
                for s in range(16):
                    emit_scores(u, s)
                    if s == 1:
                        emit_zero(u)
                    if fq and s % 3 == 2:
                        fq.pop(0)()
                    if s > 0:
                        emit_pv(u, s - 1)
                for f in fq:
                    f()
                emit_pv(u, 15)
                emit_norm(u)

            def emit_transposes_j(tb, j, ctxnT):
                """Transpose the e-columns of head-pair j (128 of 512) of all
                four t-chunks of tb into ctxnT[:, j, :] — runs right after
                unit (j, tb)'s normalize, keeping the tail off the end."""
                tr = proj_ps.tile([128, 512], BF16, tag="proj")
                for tc in range(4):
                    nc.tensor.transpose(
                        tr[:, tc * 128 : (tc + 1) * 128],
                        ctxn_sb[tb, tc][:, j * 128 : (j + 1) * 128],
                        ident,
                    )
                nc.vector.tensor_copy(out=ctxnT[:, j, :], in_=tr)

            def emit_og(tb, o):
                """One e_out-chunk of the output projection for t-block tb
                (single psum group) — sized to thread into a window's fill
                slots without starving the exp stream."""
                tsl = slice(tb * 512, (tb + 1) * 512)
                ps = proj_ps.tile([128, 512], F32, tag="proj")
                for c in range(4):
                    nc.tensor.matmul(
                        ps,
                        lhsT=woT[:, c, o * 128 : (o + 1) * 128],
                        rhs=ctxnT[tb][:, c, :],
                        start=(c == 0),
                        stop=(c == 3),
                    )
                osb = osbp.tile([128, 512], BF16, tag="osb")
                nc.vector.tensor_copy(out=osb, in_=ps)
                nc.sync.dma_start(out=outd[o * 128 : (o + 1) * 128, tsl], in_=osb)

            ctxnT = {}
            for tb in range(4):
                for tc in range(4):
                    ctxn_sb[tb, tc] = normp.tile(
                        [128, EL], BF16, tag="ctxn", name=f"ctxn{tb}_{tc}"
                    )
                ctxnT[tb] = normp.tile(
                    [128, 4, 512], BF16, tag="ctxnT", bufs=2, name=f"ctxnT{tb}"
                )

            # ---- tb0: one-behind-PV pipeline over 5 windows --------------
            # Window w streams scores/exp of unit w while PV of unit w-1
            # absorbs the v-projection (w=1) and k-row JIT fills; a short
            # PV-only flush window closes the block.  This keeps ACT (exp)
            # streaming from ~15us even though PE has ~100us of projection
            # work to retire in tb0.
            qc1 = [None]
            vc_pre = [None]

            def load_qc1():
                qc1[0] = load_x_chunk(qtd, 1)

            for w in range(5):
                u_sc = w if w < 4 else None
                u_pv = w - 1 if w >= 1 else None
                fills = []
                if w < 3:
                    fills.append(qg_thunk(w + 1))
                if w == 3:
                    fills.append(load_qc1)
                if w == 4:
                    # only tb1-unit0's own q-group is critical here; the
                    # rest of tb1's q-groups and tb0's transposes ride
                    # inside tb1's ACT-bound windows, keeping this
                    # (exp-less) flush window as short as possible
                    fills.append(
                        lambda: proj_qk_group(qc1[0], qpT, bq_sb, wqT, 0, 1)
                    )
                for s in range(16):
                    if u_sc is not None:
                        if s % 4 == 0 and (w > 0 or s > 0):
                            # k-row c=w, t-chunk s//4 just-in-time before the
                            # scores that consume it
                            proj_qk_group(
                                ktf[:, :, (s // 4) * 512 : (s // 4 + 1) * 512],
                                kpT, bk_sb, wkT, w, s // 4,
                            )
                        emit_scores(u_sc, s)
                    if s == (1 if u_sc is not None else 0) and u_pv is not None:
                        emit_zero(u_pv)
                    if w == 1:
                        proj_v_schunk(vcs[s // 4], s)
                    if u_pv is not None and s > 0:
                        emit_pv(u_pv, s - 1)
                    if fills and s % 3 == 2:
                        fills.pop(0)()
                for f in fills:
                    f()
                if u_pv is not None:
                    emit_pv(u_pv, 15)
                    emit_norm(u_pv)

            # ---- tb1..3: steady state, fills spread evenly ---------------
            # Previous block's transposes + output projection and the next
            # block's q projection are distributed 3-4 thunks per window so
            # no single window's PE overruns its ~6us of slack (bunching 8
            # out-proj groups into one window was starving exp).
            qc_t = [qc1[0]]

            def qload_thunk(tbn):
                def f():
                    qc_t[0] = load_x_chunk(qtd, tbn)
                return f

            def qgt_thunk(c, tbn):
                def f():
                    proj_qk_group(qc_t[0], qpT, bq_sb, wqT, c, tbn)
                return f

            def tr_thunk(tbx, jj):
                return lambda: emit_transposes_j(tbx, jj, ctxnT[tbx])

            def og_thunk(tbx, o):
                return lambda: emit_og(tbx, o)

            for tb in range(1, 4):
                p = tb - 1  # whose transposes/out-proj ride in this block
                fills_by_j = {
                    0: [tr_thunk(p, jj) for jj in range(3)]
                    + [tr_thunk(p, 3), og_thunk(p, 0)],
                    1: [og_thunk(p, 1), og_thunk(p, 2), og_thunk(p, 3)],
                    2: [og_thunk(p, 4), og_thunk(p, 5), og_thunk(p, 6)],
                    3: [og_thunk(p, 7)],
                }
                if tb == 1:
                    # tb1's remaining q-groups, one unit ahead of their
                    # consumers (the flush window only did c0).  Bound to
                    # qc1's tile directly: qc_t[0] is rebound to tb2's chunk
                    # by the interleaved qload fill.
                    def qg1_thunk(c):
                        return lambda: proj_qk_group(
                            qc1[0], qpT, bq_sb, wqT, c, 1
                        )

                    fills_by_j[0].insert(3, qg1_thunk(1))
                    fills_by_j[1].append(qg1_thunk(2))
                    fills_by_j[2].append(qg1_thunk(3))
                if tb < 3:
                    fills_by_j[1].append(qload_thunk(tb + 1))
                    fills_by_j[2].append(qgt_thunk(0, tb + 1))
                    fills_by_j[3].extend(
                        qgt_thunk(c, tb + 1) for c in (1, 2, 3)
                    )
                else:
                    # tb3's own transposes ride one unit behind their norms
                    fills_by_j[1].append(tr_thunk(3, 0))
                    fills_by_j[2].append(tr_thunk(3, 1))
                    fills_by_j[3].append(tr_thunk(3, 2))
                for j in range(4):
                    emit_unit(j, tb, fills=fills_by_j[j])

            emit_transposes_j(3, 3, ctxnT[3])
            for o in range(8):
                emit_og(3, o)

    legalize_waits(nc)
    return nc


def _make_in_maps(inputs):
    import ml_dtypes

    bf16 = ml_dtypes.bfloat16
    q, k, v = inputs["q"], inputs["k"], inputs["v"]
    f32 = np.float32

    def tcast(a):
        return np.ascontiguousarray(np.asarray(a, dtype=f32).T.astype(bf16))

    # per-batch transposed activations, shared by the two cores of a batch
    qT = [tcast(q[b]) for b in range(B)]
    kT = [tcast(k[b]) for b in range(B)]
    vT = [tcast(v[b]) for b in range(B)]
    Wq, Wk, Wv, Wo = inputs["Wq"], inputs["Wk"], inputs["Wv"], inputs["Wo"]
    in_maps = []
    for c in range(N_CORES):
        b, hh = c // 2, c % 2
        esl = slice(hh * EL, (hh + 1) * EL)
        in_maps.append(
            {
                "qT": qT[b],
                "kT": kT[b],
                "vT": vT[b],
                "wqT": tcast(np.asarray(Wq, dtype=f32)[esl]),
                "wkT": tcast(np.asarray(Wk, dtype=f32)[esl]),
                "wvT": tcast(np.asarray(Wv, dtype=f32)[esl]),
                "woT": tcast(np.asarray(Wo, dtype=f32)[:, esl]),
                "bq": np.ascontiguousarray(inputs["bq"][esl], dtype=f32),
                "bk": np.ascontiguousarray(inputs["bk"][esl], dtype=f32),
                "bv": np.ascontiguousarray(inputs["bv"][esl], dtype=f32),
            }
        )
    return in_maps


def _gather(results, bo):
    out = np.empty((B, T, E), dtype=np.float32)
    for b in range(B):
        acc = (
            results[2 * b]["outT"].astype(np.float32).T
            + results[2 * b + 1]["outT"].astype(np.float32).T
        )
        out[b] = acc + bo[None, :]
    return out


def run(inputs, **spmd_kwargs):
    if "nc" not in _CACHED:
        _CACHED["nc"] = build_program()
    nc = _CACHED["nc"]
    in_maps = _make_in_maps(inputs)
    res = run_bass_kernel_spmd(nc, in_maps, core_ids=list(range(N_CORES)), **spmd_kwargs)
    out = _gather(res.results, np.asarray(inputs["bo"], dtype=np.float32))
    return out, res


def kernel(**inputs) -> np.ndarray:
    out, _ = run(inputs)
    return out


# revision 55
# speedup vs baseline: 1.0090x; 1.0090x over previous
"""Multi-head attention (B=4, T=S=2048, E=1024, H=16) on 8 trn2 NeuronCores.

Sharding: core c handles batch b = c // 2 and head-half hh = c % 2
(8 of 16 heads).  Each core computes its heads' Q/K/V projections,
attention, and a partial output projection (contraction over its 512
e-dims).  The host sums the two partial outputs per batch and adds bo.

Key layout choices:
 - The host passes PRE-TRANSPOSED, PRE-CAST-bf16 activations and
   weights (q.T, k.T, v.T, Wq_slice.T, ...), so no on-chip transposes
   of x or W are needed and DMA bytes are halved: loads land directly
   in the [e_in partitions, t] layout the projections consume.
 - scores.T = kp @ qp.T is computed per head as [s, t] tiles
   (s on partitions), exp'd on ACT into bf16 pt tiles.  exp is the
   single largest engine load (~266us on ACT); the whole schedule is
   built to keep it streaming.
 - PV runs in the "natural" orientation: ctx[t, hd] = sum_s
   pt[s, t] * vp[s, hd], i.e. lhsT = pt (stationary), rhs = vp
   with an extra ones-column producing the softmax denominator in
   column 64.  Output columns per matmul are 65 instead of 512,
   which is ~2x fewer PE cycles for the PV stage.  The 8 (head,
   t-chunk) accumulation regions share PSUM banks, so the tile is
   zeroed via single-group ones⊗zero matmuls and PV runs start=False
   (a start=True per region would clear the whole bank's has_written
   and wipe sibling regions — hardware-verified failure mode).
 - Normalization is a per-partition (per-t) reciprocal multiply on
   DVE, no cross-partition broadcast needed.
 - ctx is transposed back (PE transposes) only for the tiny
   [2048 x 512] normalized context, feeding the output projection.
 - t-block 0 runs a one-behind-PV software pipeline (scores of unit w
   stream while PV of unit w-1 + v-projection + k-row fills retire on
   PE), so exp starts ~15us in; t-blocks 1-3 run same-unit PV with the
   previous block's ctx-transposes + output projection and the next
   block's q-projection emitted into their ACT-bound windows.
"""

import numpy as np

import concourse.bass as bass
import concourse.mybir as mybir
import concourse.tile as tile
from concourse.bass_utils import run_bass_kernel_spmd
from concourse.masks import make_identity

F32 = mybir.dt.float32
BF16 = mybir.dt.bfloat16

B, T, E = 4, 2048, 1024
H = 16  # global heads
HL = 8  # heads per core (local)
HD = 64  # head dim
EL = HL * HD  # 512, e-dims per core
N_CORES = 8

_CACHED = {}


def legalize_waits(nc, cap=1):
    """Hoist semaphore waits so no instruction carries more than `cap`.

    The cayman 64B ISA instruction format has a single wait slot
    (NEURON_ISA_TPB_EVENTS); this container's walrus rejects instructions
    with more attached waits ("Too many sync wait commands").  Tile's sem
    assignment freely attaches several, so we split the excess onto
    standalone InstEventSemaphore carriers (exactly what raw-bass
    wait_ge emits) on the same engine, immediately before.
    """
    import bass_rust

    # Pass 1: statically-known final value of every semaphore (sum of all
    # attached increments) — needed to replace the tail RANGE_CLEAR (an
    # InstISA opcode this walrus can't codegen) with sem-dec updates.
    totals = {}
    names = {}
    for f in nc.m.functions:
        for bb in f.blocks:
            for ins in bb.instructions:
                si = ins.sync_info
                if si is None:
                    continue
                for u in si.on_update or []:
                    if u.sync_type == "semaphore":
                        sign = 1 if u.update_mode in ("sem-inc", "sem-add-imm") else -1
                        totals[u.id] = totals.get(u.id, 0) + sign * u.update_value
                        names[u.id] = u.ant_name

    n = 0
    for f in nc.m.functions:
        for bb in f.blocks:
            insts = bb.instructions
            out = []
            changed = False
            for ins in insts:
                if type(ins).__name__ == "InstISA" and "RANGE_CLEAR" in str(ins):
                    import re

                    m = re.search(r"range_first=(\d+) range_last=(\d+)", str(ins))
                    first, last = int(m.group(1)), int(m.group(2))
                    for sid in range(first, last + 1):
                        tot = totals.get(sid, 0)
                        if tot == 0:
                            continue
                        ev = mybir.InstEventSemaphore(name=f"I-LC{n}", ins=[], outs=[])
                        n += 1
                        ev.engine = ins.engine
                        ev.sync_info = bass_rust.SyncInfo(
                            on_wait=[],
                            on_update=[
                                bass_rust.SyncUpdate(
                                    sync_type="semaphore",
                                    id=sid,
                                    ant_name=names.get(sid, f"sem{sid}"),
                                    update_mode="sem-sub-imm",
                                    update_value=tot,
                                    update_reg=None,
                                )
                            ],
                        )
                        out.append(ev)
                    changed = True
                    continue
                si = ins.sync_info
                ws = list(si.on_wait) if (si is not None and si.on_wait) else []
                if len(ws) > cap:
                    for w in ws[: len(ws) - cap]:
                        ev = mybir.InstEventSemaphore(
                            name=f"I-LW{n}", ins=[], outs=[]
                        )
                        n += 1
                        ev.engine = ins.engine
                        ev.sync_info = bass_rust.SyncInfo(
                            on_wait=[w], on_update=[]
                        )
                        out.append(ev)
                    si.on_wait = ws[len(ws) - cap :]
                    changed = True
                out.append(ins)
            if changed:
                insts[:] = out
    return n


def build_program():
    nc = bass.Bass()

    # Activations/weights arrive pre-transposed AND pre-cast to bf16 on the
    # host: halves the DMA bytes and removes the cast from the DMA path.
    qtd = nc.declare_dram_parameter("qT", [E, T], BF16, isOutput=False)
    ktd = nc.declare_dram_parameter("kT", [E, T], BF16, isOutput=False)
    vtd = nc.declare_dram_parameter("vT", [E, T], BF16, isOutput=False)
    wqtd = nc.declare_dram_parameter("wqT", [E, EL], BF16, isOutput=False)
    wktd = nc.declare_dram_parameter("wkT", [E, EL], BF16, isOutput=False)
    wvtd = nc.declare_dram_parameter("wvT", [E, EL], BF16, isOutput=False)
    wotd = nc.declare_dram_parameter("woT", [EL, E], BF16, isOutput=False)
    bqd = nc.declare_dram_parameter("bq", [EL], F32, isOutput=False)
    bkd = nc.declare_dram_parameter("bk", [EL], F32, isOutput=False)
    bvd = nc.declare_dram_parameter("bv", [EL], F32, isOutput=False)
    outd = nc.declare_dram_parameter("outT", [E, T], BF16, isOutput=True)

    with tile.TileContext(nc, pool_alloc_mode="queue") as tc:
        with (
            tc.tile_pool(name="singles", bufs=1) as singles,
            tc.tile_pool(name="xin", bufs=4) as xin,
            tc.tile_pool(name="acts", bufs=1) as acts,
            tc.tile_pool(name="pt", bufs=1) as ptp,
            tc.tile_pool(name="norm", bufs=8) as normp,
            tc.tile_pool(name="osb", bufs=4) as osbp,
            tc.tile_pool(name="proj_ps", bufs=2, space="PSUM") as proj_ps,
            tc.tile_pool(name="sc_ps", bufs=2, space="PSUM") as sc_ps,
            tc.tile_pool(name="ctx_ps", bufs=1, space="PSUM") as ctx_ps,
        ):
            # ---------------- prologue: weights / biases / consts ----------
            ident = singles.tile([128, 128], BF16)
            make_identity(nc, ident)

            # Transposed bf16 weights, loaded directly (host pre-transposed):
            #   wqT[p, c, o] = Wq_c[o, c*128 + p]   (c,p) = e_in in [0,1024)
            wqT = singles.tile([128, 8, EL], BF16)
            wkT = singles.tile([128, 8, EL], BF16)
            wvT = singles.tile([128, 8, EL], BF16)
            # woT[p, c, o] = Wo_c[o, c*128 + p]     (c,p) = local e in [0,512)
            woT = singles.tile([128, 4, E], BF16)

            # k-path DMAs first: the whole k projection gates attention.
            # The four ramp-critical loads go out on FOUR different DGE
            # queues (gpsimd/SWDGE + 3 HWDGE engines) so their descriptor-gen
            # and latency chains overlap instead of serializing.
            nc.gpsimd.dma_start(
                out=wkT, in_=wktd.rearrange("(c p) o -> p c o", p=128)
            )
            bq_sb = singles.tile([128, 4], F32)
            bk_sb = singles.tile([128, 4], F32)
            nc.gpsimd.dma_start(out=bk_sb, in_=bkd.rearrange("(c p) -> p c", p=128))
            nc.gpsimd.dma_start(out=bq_sb, in_=bqd.rearrange("(c p) -> p c", p=128))
            ones_col = singles.tile([1, 128], BF16)
            nc.vector.memset(ones_col, 1.0)
            zero_row = singles.tile([1, 512], BF16)
            nc.vector.memset(zero_row, 0.0)
            bv_sb = singles.tile([1, EL], BF16)

            # ---------------- activations / projections --------------------
            # qpT[p, j, t] = qp[t, j*128 + p]  (pair j: head 2j at p<64)
            qpT = acts.tile([128, 4, T], BF16)
            kpT = acts.tile([128, 4, T], BF16)
            # vp_ext[p, s, h*65 + d] = vp[s*128 + p, h*64 + d]; col h*65+64 = 1
            vp_ext = acts.tile([128, 16, HL * 65], BF16)

            def load_x_chunk(xd, tb):
                """x.T [E, T] bf16 DRAM chunk tb -> bf16 SBUF [128, 8, 512]."""
                xc = xin.tile([128, 8, 512], BF16, tag="xin")
                nc.gpsimd.dma_start(
                    out=xc,
                    in_=xd[:, tb * 512 : (tb + 1) * 512].rearrange(
                        "(c p) t -> p c t", p=128
                    ),
                )
                return xc

            def proj_qk_group(xc, xpT, b_sb, wT, c, tb):
                """One (e_out chunk c, t-block tb) projection psum group."""
                ps = proj_ps.tile([128, 512], F32, tag="proj")
                for e in range(8):
                    nc.tensor.matmul(
                        ps,
                        lhsT=wT[:, e, c * 128 : (c + 1) * 128],
                        rhs=xc[:, e, :],
                        start=(e == 0),
                        stop=(e == 7),
                    )
                nc.vector.tensor_scalar_add(
                    out=xpT[:, c, tb * 512 : (tb + 1) * 512],
                    in0=ps,
                    scalar1=b_sb[:, c : c + 1],
                )

            def proj_qk_chunk(xc, xpT, b_sb, wT, tb):
                """One t-block of the q/k projection: fills xpT[:, :, tsl]."""
                for c in range(4):
                    proj_qk_group(xc, xpT, b_sb, wT, c, tb)

            def proj_v_schunk(vc, s):
                """One 128-row s-chunk of the v projection -> vp_ext[:, s, :].

                vc is the loaded vT chunk covering s; u = s % 4 indexes the
                128-slice within it.
                """
                u = s % 4
                ps = proj_ps.tile([128, 512], F32, tag="proj")
                for e in range(8):
                    nc.tensor.matmul(
                        ps,
                        lhsT=vc[:, e, u * 128 : (u + 1) * 128],
                        rhs=wvT[:, e, :],
                        start=(e == 0),
                        stop=False,
                    )
                # += ones ⊗ bv  (bias along the free dim)
                nc.tensor.matmul(ps, lhsT=ones_col, rhs=bv_sb, start=False, stop=True)
                nc.vector.memset(vp_ext[:, s, :], 1.0)
                nc.vector.tensor_copy(
                    out=vp_ext[:, s, :].rearrange("p (h x) -> p h x", x=65)[
                        :, :, 0:64
                    ],
                    in_=ps.rearrange("p (h d) -> p h d", d=64),
                )

            # k is projected c-row-wise (c = head-pair j): attention unit
            # (j, tb) only needs row c=j of kpT, so unit (0, 0) can start
            # right after row 0 — rows 1..3 are spread into tb0's units.
            # kT is loaded as one full tile (each c-row contracts over all T).
            ktf = xin.tile([128, 8, T], BF16, tag="ktf", bufs=1)
            # first k chunk rides the otherwise-idle SP/HWDGE queue so it
            # lands in parallel with wkT on the SWDGE queue
            nc.sync.dma_start(
                out=ktf[:, :, 0:512],
                in_=ktd[:, 0:512].rearrange("(c p) t -> p c t", p=128),
            )
            nc.gpsimd.dma_start(
                out=wqT, in_=wqtd.rearrange("(c p) o -> p c o", p=128)
            )
            qc = load_x_chunk(qtd, 0)
            nc.gpsimd.dma_start(
                out=ktf[:, :, 512:1024],
                in_=ktd[:, 512:1024].rearrange("(c p) t -> p c t", p=128),
            )
            nc.gpsimd.dma_start(
                out=ktf[:, :, 1024:1536],
                in_=ktd[:, 1024:1536].rearrange("(c p) t -> p c t", p=128),
            )
            nc.gpsimd.dma_start(
                out=ktf[:, :, 1536:2048],
                in_=ktd[:, 1536:2048].rearrange("(c p) t -> p c t", p=128),
            )
            nc.gpsimd.dma_start(
                out=wvT, in_=wvtd.rearrange("(c p) o -> p c o", p=128)
            )
            nc.gpsimd.dma_start(out=bv_sb, in_=bvd.rearrange("(o e) -> o e", o=1))
            # v chunks: chunk 0 early (its projection fills the ramp's
            # PE-idle gap); the rest needed from window 1 (~30us in)
            vcs = [load_x_chunk(vtd, i) for i in range(4)]

            # Minimal pre-attention PE work: one k-group and one q-group —
            # exactly what scores (j=0, tb=0, s-chunk 0) needs.  Everything
            # else rides just-in-time inside tb0's units.
            proj_qk_group(ktf[:, :, 0:512], kpT, bk_sb, wkT, 0, 0)
            proj_qk_group(qc, qpT, bq_sb, wqT, 0, 0)
            nc.gpsimd.dma_start(
                out=woT, in_=wotd.rearrange("(c p) o -> p c o", p=128)
            )

            def kg_thunk(c, tbk):
                def f():
                    proj_qk_group(
                        ktf[:, :, tbk * 512 : (tbk + 1) * 512],
                        kpT, bk_sb, wkT, c, tbk,
                    )
                return f

            def qg_thunk(c):
                def f():
                    proj_qk_group(qc, qpT, bq_sb, wqT, c, 0)
                return f

            # ---------------- attention ----------------------------------
            # normalized ctx, natural layout, per t-chunk of 128:
            # ctxn[t', e_local] for t = tb*512 + tc*128 + t'
            # assembled across the 4 j-units of a t-block.
            ctxn_sb = {}

            v_loaded = [None]

            pt_store = {}  # u -> list of 16 pt tiles
            ctx_store = {}  # u -> ctx psum tile

            def emit_zero(u):
                """Alloc unit u's ctx tile and zero it.  The 8 (head,
                t-chunk) PV accumulation regions interleave within its two
                banks; a start=True on hardware clears has_written for the
                WHOLE bank, wiping sibling regions, so instead: zero each
                bank with a single-group ones⊗zero matmul and run every PV
                matmul in accumulate mode (correct for any has_written
                state: bit=1 accumulates onto 0, bit=0 overwrites)."""
                ctx = ctx_ps.tile([128, 2, 512], F32, tag="ctx", name=f"ctx{u}")
                ctx_store[u] = ctx
                for hh in range(2):
                    # only the 4x65-column accumulation regions need zeroing
                    nc.tensor.matmul(
                        ctx[:, hh, :].rearrange("p (tc x) -> p tc x", x=128)[
                            :, :, 0:65
                        ],
                        lhsT=ones_col,
                        rhs=zero_row[:, 0:260],
                        start=True,
                        stop=True,
                    )

            def emit_scores(u, s):
                tb, j = divmod(u, 4)
                tsl = slice(tb * 512, (tb + 1) * 512)
                sc = sc_ps.tile([128, 1024], F32, tag="sc")
                nc.tensor.matmul(
                    sc[:, 0:512],
                    lhsT=kpT[0:64, j, s * 128 : (s + 1) * 128],
                    rhs=qpT[0:64, j, tsl],
                    start=True,
                    stop=True,
                )
                nc.tensor.matmul(
                    sc[:, 512:1024],
                    lhsT=kpT[64:128, j, s * 128 : (s + 1) * 128],
                    rhs=qpT[64:128, j, tsl],
                    start=True,
                    stop=True,
                )
                pt = ptp.tile(
                    [128, 1024], BF16, tag="pt", bufs=18, name=f"pt{u}_{s}"
                )
                nc.scalar.activation(
                    out=pt,
                    in_=sc,
                    func=mybir.ActivationFunctionType.Exp,
                    scale=0.125,
                )
                pt_store.setdefault(u, {})[s] = pt

            def emit_pv(u, s):
                tb, j = divmod(u, 4)
                ctx = ctx_store[u]
                pt = pt_store[u].pop(s)
                for hh in range(2):
                    for tc in range(4):
                        nc.tensor.matmul(
                            ctx[:, hh, tc * 128 : tc * 128 + 65],
                            lhsT=pt[
                                :, hh * 512 + tc * 128 : hh * 512 + (tc + 1) * 128
                            ],
                            rhs=vp_ext[
                                :, s, (2 * j + hh) * 65 : (2 * j + hh + 1) * 65
                            ],
                            start=False,
                            stop=(s == 15),
                            skip_group_check=True,
                        )

            def emit_norm(u):
                # normalize: column tc*128+64 of ctx[:, hh, :] holds denoms
                tb, j = divmod(u, 4)
                ctx = ctx_store.pop(u)
                recip = normp.tile(
                    [128, 2, 4, 1], F32, tag="recip", bufs=2, name=f"recip{u}"
                )
                nc.vector.reciprocal(
                    out=recip,
                    in_=ctx.rearrange("p h (tc x) -> p h tc x", x=128)[
                        :, :, :, 64:65
                    ],
                )
                for tc in range(4):
                    for hh in range(2):
                        hl = 2 * j + hh
                        nc.vector.tensor_scalar_mul(
                            out=ctxn_sb[tb, tc][:, hl * 64 : (hl + 1) * 64],
                            in0=ctx[:, hh, tc * 128 : tc * 128 + 64],
                            scalar1=recip[:, hh, tc, :],
                        )

            def emit_unit(j, tb, fills=()):
                """Scores+PV streaming with PV one s-step behind: zero(u)
                (which waits on norm(u-1)'s ctx slot, fresh DVE output) is
                emitted after the first scores, so the boundary stall hides
                behind the exp stream instead of blocking PE head-of-line."""
                u = 4 * tb + j
                fq = list(fills)
                for s in range(16):
                    emit_scores(u, s)
                    if s == 1:
                        emit_zero(u)
                    if fq and s % 3 == 2:
                        fq.pop(0)()
                    if s > 0:
                        emit_pv(u, s - 1)
                for f in fq:
                    f()
                emit_pv(u, 15)
                emit_norm(u)

            def emit_transposes_j(tb, j, ctxnT):
                """Transpose the e-columns of head-pair j (128 of 512) of all
                four t-chunks of tb into ctxnT[:, j, :] — runs right after
                unit (j, tb)'s normalize, keeping the tail off the end."""
                tr = proj_ps.tile([128, 512], BF16, tag="proj")
                for tc in range(4):
                    nc.tensor.transpose(
                        tr[:, tc * 128 : (tc + 1) * 128],
                        ctxn_sb[tb, tc][:, j * 128 : (j + 1) * 128],
                        ident,
                    )
                nc.vector.tensor_copy(out=ctxnT[:, j, :], in_=tr)

            def emit_og(tb, o):
                """One e_out-chunk of the output projection for t-block tb
                (single psum group) — sized to thread into a window's fill
                slots without starving the exp stream."""
                tsl = slice(tb * 512, (tb + 1) * 512)
                ps = proj_ps.tile([128, 512], F32, tag="proj")
                for c in range(4):
                    nc.tensor.matmul(
                        ps,
                        lhsT=woT[:, c, o * 128 : (o + 1) * 128],
                        rhs=ctxnT[tb][:, c, :],
                        start=(c == 0),
                        stop=(c == 3),
                    )
                osb = osbp.tile([128, 512], BF16, tag="osb")
                nc.vector.tensor_copy(out=osb, in_=ps)
                nc.sync.dma_start(out=outd[o * 128 : (o + 1) * 128, tsl], in_=osb)

            ctxnT = {}
            for tb in range(4):
                for tc in range(4):
                    ctxn_sb[tb, tc] = normp.tile(
                        [128, EL], BF16, tag="ctxn", name=f"ctxn{tb}_{tc}"
                    )
                ctxnT[tb] = normp.tile(
                    [128, 4, 512], BF16, tag="ctxnT", bufs=2, name=f"ctxnT{tb}"
                )

            # ---- tb0: one-behind-PV pipeline over 5 windows --------------
            # Window w streams scores/exp of unit w while PV of unit w-1
            # absorbs the v-projection (w=1) and k-row JIT fills; a short
            # PV-only flush window closes the block.  This keeps ACT (exp)
            # streaming from ~15us even though PE has ~100us of projection
            # work to retire in tb0.
            qc1 = [None]
            vc_pre = [None]

            def load_qc1():
                qc1[0] = load_x_chunk(qtd, 1)

            for w in range(5):
                u_sc = w if w < 4 else None
                u_pv = w - 1 if w >= 1 else None
                fills = []
                if w < 3:
                    fills.append(qg_thunk(w + 1))
                if w == 3:
                    fills.append(load_qc1)
                if w == 4:
                    # only tb1-unit0's own q-group is critical here; the
                    # rest of tb1's q-groups and tb0's transposes ride
                    # inside tb1's ACT-bound windows, keeping this
                    # (exp-less) flush window as short as possible
                    fills.append(
                        lambda: proj_qk_group(qc1[0], qpT, bq_sb, wqT, 0, 1)
                    )
                for s in range(16):
                    if u_sc is not None:
                        if s % 4 == 0 and (w > 0 or s > 0):
                            # k-row c=w, t-chunk s//4 just-in-time before the
                            # scores that consume it
                            proj_qk_group(
                                ktf[:, :, (s // 4) * 512 : (s // 4 + 1) * 512],
                                kpT, bk_sb, wkT, w, s // 4,
                            )
                        emit_scores(u_sc, s)
                    if s == (1 if u_sc is not None else 0) and u_pv is not None:
                        emit_zero(u_pv)
                    if w == 1:
                        proj_v_schunk(vcs[s // 4], s)
                    if u_pv is not None and s > 0:
                        emit_pv(u_pv, s - 1)
                    if fills and s % 3 == 2:
                        fills.pop(0)()
                for f in fills:
                    f()
                if u_pv is not None:
                    emit_pv(u_pv, 15)
                    emit_norm(u_pv)

            # ---- tb1..3: steady state, fills spread evenly ---------------
            # Previous block's transposes + output projection and the next
            # block's q projection are distributed 3-4 thunks per window so
            # no single window's PE overruns its ~6us of slack (bunching 8
            # out-proj groups into one window was starving exp).
            qc_t = [qc1[0]]

            def qload_thunk(tbn):
                def f():
                    qc_t[0] = load_x_chunk(qtd, tbn)
                return f

            def qgt_thunk(c, tbn):
                def f():
                    proj_qk_group(qc_t[0], qpT, bq_sb, wqT, c, tbn)
                return f

            def tr_thunk(tbx, jj):
                return lambda: emit_transposes_j(tbx, jj, ctxnT[tbx])

            def og_thunk(tbx, o):
                return lambda: emit_og(tbx, o)

            for tb in range(1, 4):
                p = tb - 1  # whose transposes/out-proj ride in this block
                fills_by_j = {
                    0: [tr_thunk(p, jj) for jj in range(3)]
                    + [tr_thunk(p, 3), og_thunk(p, 0)],
                    1: [og_thunk(p, 1), og_thunk(p, 2), og_thunk(p, 3)],
                    2: [og_thunk(p, 4), og_thunk(p, 5), og_thunk(p, 6)],
                    3: [og_thunk(p, 7)],
                }
                if tb == 1:
                    # tb1's remaining q-groups, one unit ahead of their
                    # consumers (the flush window only did c0).  Bound to
                    # qc1's tile directly: qc_t[0] is rebound to tb2's chunk
                    # by the interleaved qload fill.
                    def qg1_thunk(c):
                        return lambda: proj_qk_group(
                            qc1[0], qpT, bq_sb, wqT, c, 1
                        )

                    fills_by_j[0].insert(3, qg1_thunk(1))
                    fills_by_j[1].append(qg1_thunk(2))
                    fills_by_j[2].append(qg1_thunk(3))
                if tb < 3:
                    fills_by_j[1].append(qload_thunk(tb + 1))
                    fills_by_j[2].append(qgt_thunk(0, tb + 1))
                    fills_by_j[3].extend(
                        qgt_thunk(c, tb + 1) for c in (1, 2, 3)
                    )
                else:
                    # tb3's own transposes ride one unit behind their norms
                    fills_by_j[1].append(tr_thunk(3, 0))
                    fills_by_j[2].append(tr_thunk(3, 1))
                    fills_by_j[3].append(tr_thunk(3, 2))
                for j in range(4):
                    emit_unit(j, tb, fills=fills_by_j[j])

            emit_transposes_j(3, 3, ctxnT[3])
            for o in range(8):
                emit_og(3, o)

    legalize_waits(nc)
    return nc


def _make_in_maps(inputs):
    import ml_dtypes

    bf16 = ml_dtypes.bfloat16
    q, k, v = inputs["q"], inputs["k"], inputs["v"]
    f32 = np.float32

    def tcast(a):
        return np.ascontiguousarray(np.asarray(a, dtype=f32).T.astype(bf16))

    # per-batch transposed activations, shared by the two cores of a batch
    qT = [tcast(q[b]) for b in range(B)]
    kT = [tcast(k[b]) for b in range(B)]
    vT = [tcast(v[b]) for b in range(B)]
    Wq, Wk, Wv, Wo = inputs["Wq"], inputs["Wk"], inputs["Wv"], inputs["Wo"]
    in_maps = []
    for c in range(N_CORES):
        b, hh = c // 2, c % 2
        esl = slice(hh * EL, (hh + 1) * EL)
        in_maps.append(
            {
                "qT": qT[b],
                "kT": kT[b],
                "vT": vT[b],
                "wqT": tcast(np.asarray(Wq, dtype=f32)[esl]),
                "wkT": tcast(np.asarray(Wk, dtype=f32)[esl]),
                "wvT": tcast(np.asarray(Wv, dtype=f32)[esl]),
                "woT": tcast(np.asarray(Wo, dtype=f32)[:, esl]),
                "bq": np.ascontiguousarray(inputs["bq"][esl], dtype=f32),
                "bk": np.ascontiguousarray(inputs["bk"][esl], dtype=f32),
                "bv": np.ascontiguousarray(inputs["bv"][esl], dtype=f32),
            }
        )
    return in_maps


def _gather(results, bo):
    out = np.empty((B, T, E), dtype=np.float32)
    for b in range(B):
        acc = (
            results[2 * b]["outT"].astype(np.float32).T
            + results[2 * b + 1]["outT"].astype(np.float32).T
        )
        out[b] = acc + bo[None, :]
    return out


def run(inputs, **spmd_kwargs):
    if "nc" not in _CACHED:
        _CACHED["nc"] = build_program()
    nc = _CACHED["nc"]
    in_maps = _make_in_maps(inputs)
    res = run_bass_kernel_spmd(nc, in_maps, core_ids=list(range(N_CORES)), **spmd_kwargs)
    out = _gather(res.results, np.asarray(inputs["bo"], dtype=np.float32))
    return out, res


def kernel(**inputs) -> np.ndarray:
    out, _ = run(inputs)
    return out
